# revision 98
# baseline (speedup 1.0000x reference)
"""Multi-head attention (B=4, S=2048, D=1024, H=16, causal) on 8 Trainium2 cores.

Sharding: core c -> (batch b = c//2, head-group hg = c%2, 8 heads each).
Each core computes its 8 heads' attention for its batch element plus the
partial output projection against the corresponding 512 columns of Wo.
Host sums the two partial projections per batch element and adds bo.

v2 schedule (217.6us vs the v1 298us kernel), driven by the TimelineSim cost
model (matmul = out-free-size cycles @2.4GHz regardless of contraction
width; ACT = free-size @1.2GHz + ~185ns/instr; DMA: one global serial
HWDGE ~625ns/instr, one global transfer device ~360GB/s, 900ns completion
sem). Main ideas:

  * PV is FLIPPED: stationary = p q-chunk [128k x 128q], moving =
    V_aug [128k x 65] -> accumulates o^T[q, 65] per (head, q-chunk) in
    PSUM. Cost 65 cyc/chunk-head vs 512/head in v1: 70.7k vs 139.3k
    cycles. The softmax denominator arrives as output column 64 (ones
    column of V); normalization is a per-partition reciprocal +
    tensor_scalar_mul during the PSUM->SBUF copy (replacing v1's
    reciprocal[1,512] + gpsimd partition_broadcast + [64,512] mul), and
    an SBUF->SBUF XBAR DMA transpose per chunk pair restores the [d', q]
    layout needed by the z projection -- no PE or DVE cycles.
  * PV lags the scores by 3 k-tiles (4 in the deeper t2/t3 sections,
    where the exp stream runs hotter) so the per-k-tile critical cycle
    never routes through the exp stream (score -> exp -> PV would bound
    the beat at exp time + 2 sem hops ~1.5us vs ~1.1us of PE work); the
    per-ki order is scores -> lagged PV -> filler.
  * all projection (Q/K) and output (z) work is diced into ~2-matmul
    micro-thunks sharing one PSUM accumulation group, popped one per
    attention k-tile from a FIFO whose head unit must exhaust before the
    next starts (keeps at most one aux group mid-flight). Q/K units are
    pushed at the queue front (they gate the next section's scores); the
    last z(t2) row-blocks are reserved per-section for the trailing t3
    sections, where the exp stream (~15.4us/section vs ~13us of
    fillable PE work) is the hard wall.
  * host-side tensors are staged in SBUF-exact layouts and fetched with
    few large DMAs; the startup-critical wq/xq/wk/xk(t0) loads lead the
    queue (xq/xk quartered so the first matmuls start ~2.3us earlier),
    and the V-path loads (wv, xv) precede the remaining weights. The
    xv(t+1) fetch is deferred to mid-section to keep the serial DMA
    device free for the o-transposes at section boundaries.

PSUM banks (2KB each, 8 total): scores 2 tiles x [128,1024]f32 = 4,
pv accumulator [128,642]f32 (chunks 0-2 in bank A at cols 0/130/260,
chunk 3 in bank B at col 512; one accumulation group per bank via
first-start/last-stop flags, start zeroes the whole 2KB region) = 2,
aux micro-thunk pool bufs=2 = 2.

Device-side layouts (host-prepared, all bf16 except biases):
  xq/xk [128, DT, S]      -- x.T as (p, dt, c): [p,dt,c] = x.T[128dt+p, c]
  xv    [128, KT, DT, 128] -- kt-major so the t0 V projections unblock on
                              256KB quarter-fetches instead of the full 1MB
  wq/wk [128, NPAIR, DT, 128] -- [p,pr,dt,c] = W_part.T[128dt+p, 128pr+c]
  wv    [128, DT, DPC], wo [128, JT, D]
  bq/bk/bv [512] f32, tri [128,128] (tri[k,q]=1 iff k<=q)

Scores are computed transposed (S_T[k, q]); exp(s/8) on ScalarE with no
max subtraction (scores ~N(0,1) for this problem's inputs).
"""

import os
import sys

import numpy as np

for _p in ("/opt/trn_rl_repo", "/root/.axon_site/_ro/trn_rl_repo"):
    if os.path.isdir(_p):
        if _p not in sys.path:
            sys.path.insert(0, _p)
        break

import ml_dtypes

import concourse.bass as bass
import concourse.bacc as bacc
import concourse.tile as tile
from concourse import mybir
from concourse import bass_utils

B, S, D, H = 4, 2048, 1024, 16
HD = D // H            # 64
NCORES = 8
HPC = 8                # heads per core
DPC = 512              # d' (head dims) per core
NPAIR = 4              # head pairs per core
KT = S // 128          # 16 k-tiles
QT = S // 512          # 4 q-tiles (512 wide)
DT = D // 128          # 8 d-tiles
JT = DPC // 128        # 4 d'-tiles

F32 = mybir.dt.float32
BF16 = mybir.dt.bfloat16

_NC_CACHE = {}


def _emit(tc, debug=False, reps=1):
    nc = tc.nc

    xqd = nc.dram_tensor("xq", [128, DT, S], BF16, kind="ExternalInput").ap()
    xkd = nc.dram_tensor("xk", [128, DT, S], BF16, kind="ExternalInput").ap()
    xvd = nc.dram_tensor("xv", [128, KT, DT, 128], BF16, kind="ExternalInput").ap()
    wqd = nc.dram_tensor("wq", [128, NPAIR, DT, 128], BF16, kind="ExternalInput").ap()
    wkd = nc.dram_tensor("wk", [128, NPAIR, DT, 128], BF16, kind="ExternalInput").ap()
    wvd = nc.dram_tensor("wv", [128, DT, DPC], BF16, kind="ExternalInput").ap()
    wod = nc.dram_tensor("wo", [128, JT, D], BF16, kind="ExternalInput").ap()
    bqd = nc.dram_tensor("bq", [DPC], F32, kind="ExternalInput").ap()
    bkd = nc.dram_tensor("bk", [DPC], F32, kind="ExternalInput").ap()
    bvd = nc.dram_tensor("bv", [DPC], F32, kind="ExternalInput").ap()
    trid = nc.dram_tensor("tri", [128, 128], BF16, kind="ExternalInput").ap()
    z = nc.dram_tensor("z", [S, D], BF16, kind="ExternalOutput").ap()
    dbg = {}
    if debug:
        dbg["qT"] = nc.dram_tensor("dbg_qT", [128, NPAIR, S], BF16, kind="ExternalOutput").ap()
        dbg["kT"] = nc.dram_tensor("dbg_kT", [128, NPAIR, S], BF16, kind="ExternalOutput").ap()
        dbg["v"] = nc.dram_tensor("dbg_v", [128, KT, HPC, 65], BF16, kind="ExternalOutput").ap()
        dbg["ont"] = nc.dram_tensor("dbg_ont", [128, NPAIR, 512], BF16, kind="ExternalOutput").ap()

    from contextlib import ExitStack

    for _rep in range(reps):
      with ExitStack() as stack:
        singles = stack.enter_context(tc.tile_pool(name="singles", bufs=1))
        qkv = stack.enter_context(tc.tile_pool(name="qkv", bufs=1))

        qT_sb = qkv.tile([128, NPAIR, S], BF16)   # [d'-in-pair, pair, q]
        kT_sb = qkv.tile([128, NPAIR, S], BF16)
        # V augmented per head: cols 0:64 = V_h, col 64 = ones (softmax denom)
        v_sb = qkv.tile([128, KT, HPC, 65], BF16)

        with (
            tc.tile_pool(name="wts", bufs=1) as w_pool,
            tc.tile_pool(name="xk", bufs=2) as xk_pool,
            tc.tile_pool(name="xv", bufs=2) as xv_pool,
            tc.tile_pool(name="xq", bufs=2) as xq_pool,
            tc.tile_pool(name="p_sb", bufs=10) as p_pool,
            tc.tile_pool(name="onrm", bufs=4) as onrm_pool,
            tc.tile_pool(name="rcp", bufs=2) as rcp_pool,
            tc.tile_pool(name="o_nt", bufs=3) as o_pool,
            tc.tile_pool(name="z_sb", bufs=4) as z_pool,
            tc.tile_pool(name="score_ps", bufs=2, space="PSUM") as score_ps,
            tc.tile_pool(name="pv_ps", bufs=1, space="PSUM") as pv_ps,
            tc.tile_pool(name="aux_ps", bufs=2, space="PSUM") as aux_ps,
        ):
            wq_sb = w_pool.tile([128, NPAIR, DT, 128], BF16, tag="wq")
            wk_sb = w_pool.tile([128, NPAIR, DT, 128], BF16, tag="wk")
            wv_sb = w_pool.tile([128, DT, DPC], BF16, tag="wv")
            woT_sb = w_pool.tile([128, JT, D], BF16, tag="wo")

            xq_t = {}
            xk_t = {}
            xv_t = {}

            def fetch_qk(t):
                csl = slice(512 * t, 512 * (t + 1))
                cq = xq_pool.tile([128, DT, 512], BF16, tag="xq")
                cv = xk_pool.tile([128, DT, 512], BF16, tag="xk")
                nc.sync.dma_start(out=cq, in_=xqd[:, :, csl])
                nc.sync.dma_start(out=cv, in_=xkd[:, :, csl])
                xq_t[t] = cq
                xk_t[t] = cv

            def fetch_v(t, quarters=False):
                cv = xv_pool.tile([128, 4, DT, 128], BF16, tag="xv")
                if quarters:
                    for r in range(4):
                        nc.sync.dma_start(
                            out=cv[:, r : r + 1, :, :],
                            in_=xvd[:, 4 * t + r : 4 * t + r + 1, :, :],
                        )
                else:
                    nc.sync.dma_start(out=cv, in_=xvd[:, 4 * t : 4 * t + 4, :, :])
                xv_t[t] = cv

            # ---- startup-critical DMA order (HWDGE is a serial device):
            # wq/xq/wk/xk for (t0, pr0) first, everything else after.
            csl0 = slice(0, 512)
            xq0 = xq_pool.tile([128, DT, 512], BF16, tag="xq")
            xk0 = xk_pool.tile([128, DT, 512], BF16, tag="xk")
            nc.sync.dma_start(out=wq_sb[:, 0:1, 0:4, :], in_=wqd[:, 0:1, 0:4, :])
            nc.sync.dma_start(
                out=xq0[:, 0:2, :], in_=xqd[:, 0:2, csl0]
            )
            nc.sync.dma_start(out=wk_sb[:, 0:1, 0:4, :], in_=wkd[:, 0:1, 0:4, :])
            nc.sync.dma_start(
                out=xk0[:, 0:2, :], in_=xkd[:, 0:2, csl0]
            )
            nc.sync.dma_start(out=wq_sb[:, 0:1, 4:8, :], in_=wqd[:, 0:1, 4:8, :])
            nc.sync.dma_start(out=wk_sb[:, 0:1, 4:8, :], in_=wkd[:, 0:1, 4:8, :])
            for a in range(1, 4):
                nc.sync.dma_start(
                    out=xq0[:, 2 * a : 2 * a + 2, :], in_=xqd[:, 2 * a : 2 * a + 2, csl0]
                )
                nc.sync.dma_start(
                    out=xk0[:, 2 * a : 2 * a + 2, :], in_=xkd[:, 2 * a : 2 * a + 2, csl0]
                )
            xq_t[0] = xq0
            xk_t[0] = xk0
            # tri/bq/bk ride the Pool SWDGE path: keeps three 625ns HWDGE
            # slots out of the startup-critical wv/xv stream
            tri_sb = singles.tile([128, 128], BF16)
            nc.gpsimd.dma_start(out=tri_sb, in_=trid)
            bq_sb = singles.tile([128, JT], F32)
            nc.gpsimd.dma_start(out=bq_sb, in_=bqd.rearrange("(j p) -> p j", p=128))
            bk_sb = singles.tile([128, JT], F32)
            nc.gpsimd.dma_start(out=bk_sb, in_=bkd.rearrange("(j p) -> p j", p=128))
            nc.sync.dma_start(out=wq_sb[:, 1:2, :, :], in_=wqd[:, 1:2, :, :])
            nc.sync.dma_start(out=wk_sb[:, 1:2, :, :], in_=wkd[:, 1:2, :, :])
            nc.sync.dma_start(out=wv_sb, in_=wvd)
            fetch_v(0, quarters=True)
            nc.sync.dma_start(out=wq_sb[:, 2:4, :, :], in_=wqd[:, 2:4, :, :])
            nc.sync.dma_start(out=wk_sb[:, 2:4, :, :], in_=wkd[:, 2:4, :, :])
            nc.sync.dma_start(out=woT_sb, in_=wod)
            bvb = singles.tile([128, DPC], F32)
            nc.gpsimd.dma_start(out=bvb, in_=bvd.partition_broadcast(128))

            # exp table warmup on ScalarE (~1.3us), gated only on the tri DMA
            wrm = singles.tile([1, 1], F32)
            nc.scalar.activation(
                wrm, tri_sb[0:1, 0:1], mybir.ActivationFunctionType.Exp
            )
            # ones column: v_ones = tri_view * 0 + 1 (memset can't write bf16)
            tri_view = tri_sb.rearrange("p (a b) -> p a b", a=KT).unsqueeze(3)
            nc.vector.tensor_scalar(
                v_sb[:, :, :, 64:65],
                tri_view,
                0.0,
                1.0,
                mybir.AluOpType.mult,
                mybir.AluOpType.add,
            )

            # ---- deferred PE work units: lists of micro-thunks (~2 matmuls
            # each) sharing one PSUM group. The fill queue runs one micro per
            # attention k-tile; a unit stays at the queue head until
            # exhausted, so at most one aux group is ever mid-flight.
            def proj_micros(w_sb, x_t, dst_sb, b_sb, t, pr):
                st = {}

                def mk(lo, hi):
                    def th():
                        if lo == 0:
                            st["ps"] = aux_ps.tile(
                                [128, 512], F32, tag="aux", name="ps"
                            )
                        for dt in range(lo, hi):
                            nc.tensor.matmul(
                                st["ps"],
                                w_sb[:, pr, dt, :],
                                x_t[t][:, dt, :],
                                start=(dt == 0),
                                stop=(dt == DT - 1),
                            )
                        if hi == DT:
                            nc.vector.tensor_scalar_add(
                                dst_sb[:, pr, 512 * t : 512 * (t + 1)],
                                st["ps"],
                                b_sb[:, pr : pr + 1],
                            )
                    return th

                return [mk(0, 2), mk(2, 4), mk(4, 6), mk(6, 8)]

            def qproj_block(t, pr):
                for th in proj_micros(wq_sb, xq_t, qT_sb, bq_sb, t, pr):
                    th()

            def kproj_block(t, pr):
                for th in proj_micros(wk_sb, xk_t, kT_sb, bk_sb, t, pr):
                    th()

            def vproj_block(t, kt):
                ps = aux_ps.tile([128, 512], F32, tag="aux")
                for dt in range(DT):
                    nc.tensor.matmul(
                        ps,
                        xv_t[t][:, kt - 4 * t, dt, :],
                        wv_sb[:, dt, :],
                        start=(dt == 0),
                        stop=(dt == DT - 1),
                    )
                ps4 = ps.rearrange("p (h c) -> p h c", h=HPC)
                bv4 = bvb.rearrange("p (h c) -> p h c", h=HPC)
                nc.vector.tensor_add(v_sb[:, kt, :, 0:64], ps4, bv4)

            def z_micros(t, qs, o_nt, half, box, use_act=False):
                st = {}

                def m0():
                    idx = 2 * qs + half
                    if use_act and idx % 3 == 1:
                        # tail: the score- and pv-pool PSUM banks are dead
                        # after the last exp/flush; cycling the z
                        # accumulators across all three pools deepens the
                        # effective rotation on the closing chain
                        st["zp"] = score_ps.tile(
                            [128, 512], F32, tag="sc", name="zp"
                        )
                    elif use_act and idx % 3 == 2:
                        st["zp"] = pv_ps.tile([128, 512], F32, tag="pv", name="zp")
                    else:
                        st["zp"] = aux_ps.tile([128, 512], F32, tag="aux", name="zp")
                    for j in (0, 1):
                        nc.tensor.matmul(
                            st["zp"],
                            o_nt[:, j, 128 * qs : 128 * (qs + 1)],
                            woT_sb[:, j, 512 * half : 512 * (half + 1)],
                            start=(j == 0),
                            stop=False,
                        )

                def m1():
                    for j in (2, 3):
                        nc.tensor.matmul(
                            st["zp"],
                            o_nt[:, j, 128 * qs : 128 * (qs + 1)],
                            woT_sb[:, j, 512 * half : 512 * (half + 1)],
                            start=False,
                            stop=(j == JT - 1),
                        )
                    if half == 0:
                        box["zs"] = z_pool.tile([128, D], BF16, name="zs")
                    dst = box["zs"][:, 512 * half : 512 * (half + 1)]
                    if use_act and (qs + half) % 2 == 0:
                        nc.scalar.activation(
                            dst, st["zp"], mybir.ActivationFunctionType.Copy
                        )
                    else:
                        nc.vector.tensor_copy(dst, st["zp"])
                    r0 = 512 * t + 128 * qs
                    # one store per row-block: halves the serial HWDGE
                    # descriptor-generation slots (which collide with the
                    # o-transpose DMAs at section boundaries)
                    if half == 1:
                        nc.sync.dma_start(out=z[r0 : r0 + 128, :], in_=box["zs"])

                return [m0, m1]

            # flipped PV: out o^T[q, 65] per (head, q-chunk), p chunk stationary
            def emit_pv(pv, nki, t, pr, ki, p):
                j0 = max(0, ki - 4 * t)
                for j in range(j0, 4):
                    for h in (0, 1):
                        col = 130 * j + 65 * h if j < 3 else 512 + 65 * h
                        if j < 3:
                            start = ki == 0 and j == 0 and h == 0
                            stop = ki == 4 * t + 2 and j == 2 and h == 1
                        else:
                            start = ki == 0 and h == 0
                            stop = ki == nki - 1 and h == 1
                        nc.tensor.matmul(
                            pv[:, col : col + 65],
                            p[:, 512 * h + 128 * j : 512 * h + 128 * (j + 1)],
                            v_sb[:, ki, 2 * pr + h, 0:65],
                            start=start,
                            stop=stop,
                        )

            # deferred (deadline_sid, [micro-thunks]) units; the head unit is
            # popped one micro at a time until exhausted; force-drained at
            # each section start so a section's own Q/K precede its scores
            fillq = []

            def pop_micro():
                if not fillq:
                    return False
                dl, ths = fillq[0]
                ths.pop(0)()
                if not ths:
                    fillq.pop(0)
                return True

            def drain(sid):
                rest = []
                for dl, ths in fillq:
                    if dl <= sid:
                        for th in ths:
                            th()
                    else:
                        rest.append((dl, ths))
                fillq[:] = rest

            # Q/K for the very first pair are on the critical path
            qproj_block(0, 0)
            kproj_block(0, 0)
            tail_units = []
            sect_units = {}

            o_prev = None
            for t in range(QT):
                if t + 1 < QT:
                    fetch_qk(t + 1)
                nki = 4 * (t + 1)
                o_nt = o_pool.tile([128, NPAIR, 512], BF16)
                for pr in range(NPAIR):
                    drain(4 * t + pr)
                    if pr == 1 and t + 1 < QT:
                        fetch_v(t + 1)
                    if pr == 0 and t > 0:
                        # z(t-1) units feed the fill queue, except the last
                        # two row-blocks of z(t2): those are reserved for the
                        # trailing t3 sections, where the exp stream exceeds
                        # the attention-side PE work and no other filler
                        # remains.
                        op = o_prev
                        for qs in range(4):
                            box = {}
                            for half in (0, 1):
                                u = (4 * (t + 2), z_micros(t - 1, qs, op, half, box))
                                if t == QT - 1 and qs >= 1:
                                    sect_units.setdefault(12 + qs, []).append(u)
                                else:
                                    fillq.append(u)
                    # queue next section's Q/K projections (at the FRONT:
                    # they gate the next section's first scores)
                    nt, npr = (t, pr + 1) if pr + 1 < NPAIR else (t + 1, 0)
                    if nt < QT:
                        fillq.insert(
                            0,
                            (4 * nt + npr,
                             proj_micros(wk_sb, xk_t, kT_sb, bk_sb, nt, npr)),
                        )
                        fillq.insert(
                            0,
                            (4 * nt + npr,
                             proj_micros(wq_sb, xq_t, qT_sb, bq_sb, nt, npr)),
                        )

                    pv = pv_ps.tile([128, 642], F32, tag="pv")
                    first = t == 0 and pr == 0
                    # PV lags the scores by 2 k-tiles so the per-k-tile
                    # critical cycle never routes through the exp stream
                    # (sc -> exp -> PV would otherwise bound the beat at
                    # exp-time + 2 sem hops ~1.5us vs ~1.1us of PE work).
                    # The startup section (0,0) uses full lag: its PVs also
                    # wait on the V projections, whose x/w DMAs land last.
                    lag = nki if first else (4 if t >= 2 else 3)
                    pend = []
                    for ki in range(nki):
                        diag = ki >= 4 * t
                        off = max(0, 128 * (ki - 4 * t))
                        ksl = slice(128 * ki, 128 * (ki + 1))
                        qslo = slice(512 * t + off, 512 * (t + 1))
                        sc = score_ps.tile([128, 1024], F32, tag="sc", name="sc")
                        nc.tensor.matmul(
                            sc[:, off:512],
                            kT_sb[0:64, pr, ksl],
                            qT_sb[0:64, pr, qslo],
                            start=True,
                            stop=True,
                            tile_position=(0, 0),
                        )
                        nc.tensor.matmul(
                            sc[:, 512 + off : 1024],
                            kT_sb[64:128, pr, ksl],
                            qT_sb[64:128, pr, qslo],
                            start=True,
                            stop=True,
                            tile_position=(64, 0),
                        )
                        p = p_pool.tile([128, 1024], BF16, tag="p")
                        pv_view = p.rearrange("x (u c) -> x u c", u=2)[:, :, off:512]
                        sc_view = sc.rearrange("x (u c) -> x u c", u=2)[:, :, off:512]
                        nc.scalar.activation(
                            pv_view, sc_view,
                            mybir.ActivationFunctionType.Exp, scale=0.125,
                        )
                        new_pends = [(ki, p)]
                        if diag:
                            nc.vector.tensor_mul(
                                p[:, off : off + 128], p[:, off : off + 128], tri_sb
                            )
                            nc.vector.tensor_mul(
                                p[:, 512 + off : 512 + off + 128],
                                p[:, 512 + off : 512 + off + 128],
                                tri_sb,
                            )
                        if len(pend) >= lag:
                            emit_pv(pv, nki, t, pr, *pend.pop(0))
                        pend.extend(new_pends)
                        if pr == 0 and diag and not first:
                            vproj_block(t, ki)
                        elif not pop_micro():
                            su = sect_units.get(4 * t + pr)
                            if su:
                                su[0][1].pop(0)()
                                if not su[0][1]:
                                    su.pop(0)
                    if first:
                        # startup: pop the next head-pair's Q/K projections,
                        # then the V projections (their DMAs land last), then
                        # all four deferred PVs.
                        for _ in range(8):
                            pop_micro()
                        for ki in range(4):
                            vproj_block(0, ki)
                    # flush lagged PVs; norm each pv bank right after its
                    # accumulation group closes so the DVE normalize-copies
                    # overlap the remaining PV/flush work, then an SBUF->SBUF
                    # XBAR DMA transpose per chunk pair restores [d', q]
                    rcp = rcp_pool.tile([128, 8], F32)
                    o_nrm = onrm_pool.tile([128, 4, 2, 64], BF16)

                    def copy1(c):
                        j, h = c // 2, c % 2
                        col = 130 * j + 65 * h if j < 3 else 512 + 65 * h
                        nc.vector.tensor_scalar_mul(
                            o_nrm[:, j, h, :], pv[:, col : col + 64], rcp[:, c : c + 1]
                        )

                    def tp_dma(j):
                        nc.sync.dma_start_transpose(
                            out=o_nt[:, pr, 128 * j : 128 * (j + 1)],
                            in_=o_nrm[:, j, :, :],
                        )

                    for pe in pend:
                        emit_pv(pv, nki, t, pr, *pe)
                        if pe[0] == 4 * t + 2:  # bank-0 group closed
                            pvA = pv[:, 0:390].rearrange(
                                "p (j h c) -> p j h c", j=3, h=2
                            )
                            nc.vector.reciprocal(
                                rcp[:, 0:6]
                                .rearrange("p (j h) -> p j h", j=3)
                                .unsqueeze(3),
                                pvA[:, :, :, 64:65],
                            )
                            for c in range(6):
                                copy1(c)
                            for j in range(3):
                                tp_dma(j)
                        if pe[0] == nki - 1:  # bank-1 group closed
                            pvB = pv[:, 512:642].rearrange("p (h c) -> p h c", h=2)
                            nc.vector.reciprocal(
                                rcp[:, 6:8].unsqueeze(2), pvB[:, :, 64:65]
                            )
                            copy1(6)
                            copy1(7)
                            tp_dma(3)
                    pend = []
                if debug and t == 0:
                    nc.sync.dma_start(out=dbg["ont"], in_=o_nt)
                o_prev = o_nt
            # drain deferred work, then last q-tile's z (its PSUM->SBUF
            # copies go on the now-idle Activation engine); the reserved
            # z(t2) units fill PE while the last normalize-copies drain
            while pop_micro():
                pass
            for su in sect_units.values():
                for dl, ths in su:
                    for th in ths:
                        th()
            for dl, ths in tail_units:
                for th in ths:
                    th()
            for qs in range(4):
                box = {}
                for half in (0, 1):
                    for th in z_micros(QT - 1, qs, o_prev, half, box, use_act=True):
                        th()

        if debug:
            nc.sync.dma_start(out=dbg["qT"], in_=qT_sb)
            nc.sync.dma_start(out=dbg["kT"], in_=kT_sb)
            nc.sync.dma_start(out=dbg["v"], in_=v_sb)


def _get_nc(debug=False, reps=1):
    key = (debug, reps)
    if key not in _NC_CACHE:
        nc = bacc.Bacc(
            "TRN2", target_bir_lowering=False, debug=False, num_devices=NCORES
        )
        with tile.TileContext(nc) as tc:
            _emit(tc, debug=debug, reps=reps)
        nc.compile()
        _NC_CACHE[key] = nc
    return _NC_CACHE[key]


def _shard(inputs):
    def get(*names):
        for n in names:
            if n in inputs:
                return np.asarray(inputs[n], dtype=np.float32)
        raise KeyError(names)

    bf = ml_dtypes.bfloat16
    query = get("query")
    key_ = get("key_", "key")
    value = get("value")
    Wq, Wk, Wv, Wo = get("Wq"), get("Wk"), get("Wv"), get("Wo")
    bq, bk, bv = get("bq"), get("bk"), get("bv")
    tri = np.triu(np.ones((128, 128), dtype=np.float32)).astype(bf)

    def xlayout(x):  # [S, D] -> x.T as [128, DT, S]
        xT = np.ascontiguousarray(x.T.astype(bf))
        return np.ascontiguousarray(
            xT.reshape(DT, 128, S).transpose(1, 0, 2)
        )

    def vlayout(x):  # [S, D] -> x.T as [128, KT, DT, 128] (kt-major)
        xT = np.ascontiguousarray(x.T.astype(bf))
        return np.ascontiguousarray(
            xT.reshape(DT, 128, KT, 128).transpose(1, 2, 0, 3)
        )

    def wlayout(wT):  # W_part.T [D, DPC] -> [128, NPAIR, DT, 128]
        return np.ascontiguousarray(
            wT.reshape(DT, 128, NPAIR, 128).transpose(1, 2, 0, 3).astype(bf)
        )

    in_maps = []
    for c in range(NCORES):
        b, hg = c // 2, c % 2
        sl = slice(DPC * hg, DPC * (hg + 1))
        wvT = Wv[sl].T  # [D, DPC]
        woT = Wo[:, sl].T  # [DPC, D]
        in_maps.append(
            {
                "xq": xlayout(query[b]),
                "xk": xlayout(key_[b]),
                "xv": vlayout(value[b]),
                "wq": wlayout(Wq[sl].T),
                "wk": wlayout(Wk[sl].T),
                "wv": np.ascontiguousarray(
                    wvT.reshape(DT, 128, DPC).transpose(1, 0, 2).astype(bf)
                ),
                "wo": np.ascontiguousarray(
                    woT.reshape(JT, 128, D).transpose(1, 0, 2).astype(bf)
                ),
                "bq": np.ascontiguousarray(bq[sl]),
                "bk": np.ascontiguousarray(bk[sl]),
                "bv": np.ascontiguousarray(bv[sl]),
                "tri": tri,
            }
        )
    return in_maps


def _run(in_maps, trace=False, debug=False, **kwargs):
    nc = _get_nc(debug=debug)
    return bass_utils.run_bass_kernel_spmd(
        nc, in_maps, core_ids=list(range(len(in_maps))), trace=trace, **kwargs
    )


def _gather(results, inputs):
    bo = np.asarray(inputs["bo"], dtype=np.float32) if "bo" in inputs else 0.0
    out = np.empty((B, S, D), dtype=np.float32)
    for b in range(B):
        out[b] = (
            results[2 * b]["z"].astype(np.float32)
            + results[2 * b + 1]["z"].astype(np.float32)
            + bo
        )
    return out


def kernel(**inputs):
    in_maps = _shard(inputs)
    res = _run(in_maps)
    return _gather(res.results, inputs)
